# revision 4
# baseline (speedup 1.0000x reference)
"""MoE adapter (nn_MoEAdapter) Trainium2 Bass kernel.

Math (per token t):
    logits = x @ Wr + br                       # [*, E=8]
    gates  = softmax(logits)  (bonus constant cancels)
    top2 normalized weights w over E (w has exactly 2 nonzeros)
    out    = sum_e w_e * ( relu(x @ Wd_e + bd_e) @ Wu_e + bu_e )

Key identities exploited (bd == 0 and bu == 0 in this model):
  * E*R = 8*16 = 128, so all 8 rank-16 experts fuse into single GEMMs:
        h   = relu(x @ Wd_all)        Wd_all: [D, 128]
        out = (w_expanded * h) @ Wu_all,  Wu_all: [128, D]
  * top-2 + renormalized softmax needs only (max1, max2) per token:
        w_e = 1[l_e >= max2] * exp(l_e - max1) / sum(masked exp)

Distribution: data-parallel over the 8192 tokens across 8 NeuronCores
(1024 tokens/core); the tiny expert weights are replicated.

Numerics: all big GEMMs run in fp16 with fp32 PSUM accumulation.  x is
split hi/lo into two fp16 streams on the host (hi = fp16(x),
lo = (x-hi)*64) which gives ~fp29-equivalent precision where needed.
The router is computed in 3 passes (hi@Wr_hi + lo@Wr_hi/64 and a
separately-scaled hi@((Wr-Wr_hi)*256) correction) so that the worst-case
logit error (~4e-6) is far below the minimum top-2/top-3 gap (~3.9e-5)
=> bit-identical expert selection vs the fp32 reference.  The adapter
path (h and out GEMMs) runs 1-pass fp16: end-to-end rel err ~4e-4.

The kernel works on a host-transposed view of x (x^T, [D, tokens]) so
that the contraction dim D lands on SBUF partitions for every GEMM; the
output is produced in natural [tokens, D] layout.
"""

import numpy as np

# ---- problem constants (hardcoded per contract) ----
B, T, D, E, R = 2, 4096, 2048, 8, 16
BT = B * T                # 8192 tokens
NCORES = 8
TC = BT // NCORES         # 1024 tokens per core
MACRO = 512               # tokens per macro tile (one PSUM bank of fp32)
NMACRO = TC // MACRO      # 2
SUB = 128                 # tokens per sub tile (PE stationary width)
NSUB = MACRO // SUB       # 4
KC = D // 128             # 16 contraction chunks
ER = E * R                # 128 fused adapter width
LO_SCALE = 64.0           # x-lo stream stored as (x-hi)*64 (keeps fp16 normal)
WRL_SCALE = 256.0         # Wr-lo stored as (Wr-fp16(Wr))*256
NEG_BIG = -1.0e30

_CACHE = {}


def _split_multi_waits(nc):
    """This container's walrus rejects instructions carrying more than one
    sem-wait.  Hoist excess waits onto same-engine NOPs inserted just before
    the instruction (engine program order makes this equivalent)."""
    import concourse.mybir as mybir

    n_split = 0
    for f in nc.m.functions:
        for bb in f.blocks:
            insts = list(bb.instructions)
            out = []
            changed = False
            for ins in insts:
                si = ins.sync_info
                if si is not None and len(si.on_wait) > 1:
                    waits = list(si.on_wait)
                    for j, w in enumerate(waits[:-1]):
                        nop = mybir.InstNoOp(
                            name=f"{ins.name}-wsplit{j}", engine=ins.engine
                        )
                        nop.sync_info = mybir.SyncInfo(on_wait=[w], on_update=[])
                        out.append(nop)
                        n_split += 1
                    ins.sync_info = mybir.SyncInfo(
                        on_wait=[waits[-1]], on_update=list(si.on_update)
                    )
                    changed = True
                out.append(ins)
            if changed:
                bb.instructions = out
    return n_split


def _build_program():
    """Build the single-core SPMD Bass program (same NEFF on all 8 cores)."""
    import concourse.bass as bass
    import concourse.tile as tile
    import concourse.mybir as mybir

    dt = mybir.dt
    op = mybir.AluOpType
    AF = mybir.ActivationFunctionType

    nc = bass.Bass("TRN2", target_bir_lowering=False, debug=False, num_devices=1)

    # per-core DRAM tensors. x chunks pre-tiled on host: [p, k, t] with
    # element (d=128k+p, token t) so every DMA is contiguous per partition.
    xh_d = nc.dram_tensor("xh", [128, KC, TC], dt.float16, kind="ExternalInput").ap()
    xl_d = nc.dram_tensor("xl", [128, KC, TC], dt.float16, kind="ExternalInput").ap()
    wd_d = nc.dram_tensor("wd", [128, KC, ER], dt.float16, kind="ExternalInput").ap()
    wrh_d = nc.dram_tensor("wrh", [128, KC, E], dt.float16, kind="ExternalInput").ap()
    wrh64_d = nc.dram_tensor("wrh64", [128, KC, E], dt.float16, kind="ExternalInput").ap()
    wrl_d = nc.dram_tensor("wrl", [128, KC, E], dt.float16, kind="ExternalInput").ap()
    wu_d = nc.dram_tensor("wu", [ER, D], dt.float16, kind="ExternalInput").ap()
    brb_d = nc.dram_tensor("brb", [128, E], dt.float32, kind="ExternalInput").ap()
    ident_d = nc.dram_tensor("ident", [128, 128], dt.float32, kind="ExternalInput").ap()
    out_d = nc.dram_tensor("out", [TC, D], dt.float32, kind="ExternalOutput").ap()

    with tile.TileContext(nc) as tc:
        with (
            tc.tile_pool(name="consts", bufs=1) as cpool,
            tc.tile_pool(name="xdata", bufs=1) as xpool,
            tc.tile_pool(name="work", bufs=2) as wk,
            tc.tile_pool(name="outsb", bufs=4) as osb,
            tc.tile_pool(name="ps_l", bufs=1, space="PSUM") as ps_l,
            tc.tile_pool(name="ps_c", bufs=1, space="PSUM") as ps_c,
            tc.tile_pool(name="ps_h", bufs=2, space="PSUM") as ps_h,
            tc.tile_pool(name="ps_t", bufs=1, space="PSUM") as ps_t,
            tc.tile_pool(name="ps_w", bufs=1, space="PSUM") as ps_w,
            tc.tile_pool(name="ps_o", bufs=2, space="PSUM") as ps_o,
        ):
            # ---- constants / weights ----
            wd_sb = cpool.tile([128, KC, ER], dt.float16)
            nc.sync.dma_start(wd_sb[:], wd_d[:])
            wrh_sb = cpool.tile([128, KC, E], dt.float16)
            nc.sync.dma_start(wrh_sb[:], wrh_d[:])
            wrh64_sb = cpool.tile([128, KC, E], dt.float16)
            nc.sync.dma_start(wrh64_sb[:], wrh64_d[:])
            wrl_sb = cpool.tile([128, KC, E], dt.float16)
            nc.sync.dma_start(wrl_sb[:], wrl_d[:])
            wu_sb = cpool.tile([ER, D], dt.float16)
            nc.sync.dma_start(wu_sb[:], wu_d[:])
            brb_sb = cpool.tile([128, E], dt.float32)
            nc.sync.dma_start(brb_sb[:], brb_d[:])
            ident_sb = cpool.tile([128, 128], dt.float32)
            nc.sync.dma_start(ident_sb[:], ident_d[:])

            # ---- x streams (whole core slice resident; chunked DMAs) ----
            xh_sb = xpool.tile([128, KC, TC], dt.float16)
            xl_sb = xpool.tile([128, KC, TC], dt.float16)
            for k in range(KC):
                nc.sync.dma_start(xh_sb[:, k, :], xh_d[:, k, :])
            for k in range(KC):
                nc.sync.dma_start(xl_sb[:, k, :], xl_d[:, k, :])

            for m in range(NMACRO):
                ts = slice(m * MACRO, (m + 1) * MACRO)

                with nc.named_scope(f"router_mm_{m}"):
                    # main: hi @ Wr_hi + (lo*64) @ (Wr_hi/64)
                    psum_l = ps_l.tile([E, MACRO], dt.float32)
                    for k in range(KC):
                        nc.tensor.matmul(
                            psum_l[:], wrh_sb[:, k, :], xh_sb[:, k, ts],
                            start=(k == 0), stop=False,
                        )
                    for k in range(KC):
                        nc.tensor.matmul(
                            psum_l[:], wrh64_sb[:, k, :], xl_sb[:, k, ts],
                            start=False, stop=(k == KC - 1),
                        )
                    # correction: hi @ (Wr_lo*256), rescaled on evac
                    psum_c = ps_c.tile([E, MACRO], dt.float32)
                    for k in range(KC):
                        nc.tensor.matmul(
                            psum_c[:], wrl_sb[:, k, :], xh_sb[:, k, ts],
                            start=(k == 0), stop=(k == KC - 1),
                        )
                    corr_sb = wk.tile([E, MACRO], dt.float32)
                    nc.scalar.mul(corr_sb[:], psum_c[:], 1.0 / WRL_SCALE)
                    lT = wk.tile([E, MACRO], dt.float32)
                    nc.vector.tensor_add(lT[:], psum_l[:], corr_sb[:])

                with nc.named_scope(f"down_mm_{m}"):
                    psum_h = ps_h.tile([ER, MACRO], dt.float32)
                    for k in range(KC):
                        nc.tensor.matmul(
                            psum_h[:], wd_sb[:, k, :], xh_sb[:, k, ts],
                            start=(k == 0), stop=(k == KC - 1),
                        )

                with nc.named_scope(f"routing_{m}"):
                    # stack [E, MACRO] -> [E*NSUB, SUB] then one PE transpose
                    lT_st = wk.tile([E * NSUB, SUB], dt.float32)
                    for s in range(NSUB):
                        nc.sync.dma_start(
                            lT_st[s * E:(s + 1) * E, :], lT[:, s * SUB:(s + 1) * SUB]
                        )
                    psum_lt = ps_t.tile([128, NSUB * E], dt.float32)
                    nc.tensor.transpose(
                        psum_lt[:], lT_st[:], ident_sb[: E * NSUB, : E * NSUB]
                    )
                    # logits [tok=128, s, e] with router bias
                    l_all = wk.tile([128, NSUB, E], dt.float32)
                    brb_b = brb_sb[:].unsqueeze(1).broadcast_to([128, NSUB, E])
                    nc.vector.tensor_add(
                        l_all[:], psum_lt[:].rearrange("p (s e) -> p s e", e=E), brb_b
                    )
                    v1 = wk.tile([128, NSUB], dt.float32)
                    nc.vector.reduce_max(v1[:], l_all[:], axis=mybir.AxisListType.X)
                    v1b = v1[:].unsqueeze(-1).broadcast_to([128, NSUB, E])
                    eq = wk.tile([128, NSUB, E], dt.float32)
                    nc.vector.tensor_tensor(eq[:], l_all[:], v1b, op.is_equal)
                    lm = wk.tile([128, NSUB, E], dt.float32)
                    nc.vector.scalar_tensor_tensor(
                        lm[:], eq[:], NEG_BIG, l_all[:], op0=op.mult, op1=op.add
                    )
                    v2 = wk.tile([128, NSUB], dt.float32)
                    nc.vector.reduce_max(v2[:], lm[:], axis=mybir.AxisListType.X)
                    t1 = wk.tile([128, NSUB, E], dt.float32)
                    nc.vector.tensor_sub(t1[:], l_all[:], v1b)
                    e1 = wk.tile([128, NSUB, E], dt.float32)
                    nc.scalar.activation(e1[:], t1[:], AF.Exp)
                    v2b = v2[:].unsqueeze(-1).broadcast_to([128, NSUB, E])
                    m2 = wk.tile([128, NSUB, E], dt.float32)
                    nc.vector.tensor_tensor(m2[:], l_all[:], v2b, op.is_ge)
                    num = wk.tile([128, NSUB, E], dt.float32)
                    nc.vector.tensor_mul(num[:], e1[:], m2[:])
                    den = wk.tile([128, NSUB], dt.float32)
                    nc.vector.reduce_sum(den[:], num[:], axis=mybir.AxisListType.X)
                    rec = wk.tile([128, NSUB], dt.float32)
                    nc.vector.reciprocal(rec[:], den[:])
                    recb = rec[:].unsqueeze(-1).broadcast_to([128, NSUB, E])
                    w_all = wk.tile([128, NSUB, E], dt.float32)
                    nc.vector.tensor_mul(w_all[:], num[:], recb)

                with nc.named_scope(f"scale_up_{m}"):
                    g = wk.tile([ER, MACRO], dt.float16)
                    for s in range(NSUB):
                        # expand w over rank (free bcast), transpose to [j, t]
                        wF = wk.tile([128, E, R], dt.float32)
                        nc.vector.tensor_copy(
                            wF[:], w_all[:, s, :].unsqueeze(-1).broadcast_to([128, E, R])
                        )
                        psum_w = ps_w.tile([128, 128], dt.float32)
                        nc.tensor.transpose(
                            psum_w[:],
                            wF[:].rearrange("p e r -> p (e r)"),
                            ident_sb[:],
                        )
                        wexp = wk.tile([128, SUB], dt.float32)
                        nc.scalar.copy(wexp[:], psum_w[:])
                        # g = relu(h) * w   (w >= 0 so relu(h*w) == relu(h)*w)
                        nc.vector.scalar_tensor_tensor(
                            g[:, s * SUB:(s + 1) * SUB],
                            psum_h[:, s * SUB:(s + 1) * SUB],
                            0.0,
                            wexp[:],
                            op0=op.max,
                            op1=op.mult,
                        )
                    for s in range(NSUB):
                        row0 = m * MACRO + s * SUB
                        for dc in range(4):
                            psum_o = ps_o.tile([SUB, 512], dt.float32)
                            nc.tensor.matmul(
                                psum_o[:],
                                g[:, s * SUB:(s + 1) * SUB],
                                wu_sb[:, dc * 512:(dc + 1) * 512],
                                start=True, stop=True,
                            )
                            ob = osb.tile([SUB, 512], dt.float32)
                            if dc % 2 == 0:
                                nc.vector.tensor_copy(ob[:], psum_o[:])
                            else:
                                nc.scalar.copy(ob[:], psum_o[:])
                            nc.sync.dma_start(
                                out_d[row0:row0 + SUB, dc * 512:(dc + 1) * 512], ob[:]
                            )
    _split_multi_waits(nc)
    return nc


def _prep_inputs(x, Wr, br, Wd, Wu):
    """Host-side layout prep + sharding. Returns list of per-core in_maps."""
    f16, f32 = np.float16, np.float32
    xf = np.ascontiguousarray(x.reshape(BT, D).T)          # [D, BT] f32
    xh = xf.astype(f16)
    xl = ((xf - xh.astype(f32)) * LO_SCALE).astype(f16)

    W1 = np.ascontiguousarray(Wd.transpose(1, 0, 2).reshape(D, ER))  # [D, 128]
    wrh = Wr.astype(f16)
    wrh64 = (wrh.astype(f32) / LO_SCALE).astype(f16)
    wrl = ((Wr - wrh.astype(f32)) * WRL_SCALE).astype(f16)

    def chunkify(a, width):  # [D, width] -> [128, KC, width]
        return np.ascontiguousarray(
            a.reshape(KC, 128, width).transpose(1, 0, 2)
        )

    wd_t = chunkify(W1.astype(f16), ER)
    wrh_t = chunkify(wrh, E)
    wrh64_t = chunkify(wrh64, E)
    wrl_t = chunkify(wrl, E)
    wu_t = np.ascontiguousarray(Wu.reshape(ER, D).astype(f16))
    brb = np.ascontiguousarray(np.tile(br.astype(f32), (128, 1)))
    ident = np.eye(128, dtype=f32)

    in_maps = []
    for c in range(NCORES):
        sl = slice(c * TC, (c + 1) * TC)
        in_maps.append({
            "xh": chunkify(xh[:, sl], TC),
            "xl": chunkify(xl[:, sl], TC),
            "wd": wd_t,
            "wrh": wrh_t,
            "wrh64": wrh64_t,
            "wrl": wrl_t,
            "wu": wu_t,
            "brb": brb,
            "ident": ident,
        })
    return in_maps


def _get_program():
    if "nc" not in _CACHE:
        _CACHE["nc"] = _build_program()
    return _CACHE["nc"]


def run_on_device(in_maps, **kwargs):
    from concourse import bass_utils
    nc = _get_program()
    return bass_utils.run_bass_kernel_spmd(
        nc, in_maps, core_ids=list(range(NCORES)), **kwargs
    )


def kernel(x, Wr, br, Wd, bd, Wu, bu, **_ignored):
    x = np.asarray(x, dtype=np.float32)
    in_maps = _prep_inputs(
        x,
        np.asarray(Wr, dtype=np.float32),
        np.asarray(br, dtype=np.float32),
        np.asarray(Wd, dtype=np.float32),
        np.asarray(Wu, dtype=np.float32),
    )
    res = run_on_device(in_maps)
    out = np.concatenate([r["out"] for r in res.results], axis=0)
    return out.reshape(B, T, D)


# revision 8
# speedup vs baseline: 36.0131x; 36.0131x over previous
"""MoE adapter (nn_MoEAdapter) Trainium2 Bass kernel.

Math (per token t):
    logits = x @ Wr + br                       # [*, E=8]
    gates  = softmax(logits)  (bonus constant cancels)
    top2 normalized weights w over E (w has exactly 2 nonzeros)
    out    = sum_e w_e * ( relu(x @ Wd_e + bd_e) @ Wu_e + bu_e )

Key identities exploited (bd == 0 and bu == 0 in this model):
  * E*R = 8*16 = 128, so all 8 rank-16 experts fuse into single GEMMs:
        h   = relu(x @ Wd_all)        Wd_all: [D, 128]
        out = (w_expanded * h) @ Wu_all,  Wu_all: [128, D]
  * top-2 + renormalized softmax needs only (max1, max2) per token:
        w_e = 1[l_e >= max2] * exp(l_e - max1) / sum(masked exp)

Distribution: data-parallel over the 8192 tokens across 8 NeuronCores
(1024 tokens/core); the tiny expert weights are replicated.

Numerics: all big GEMMs run in fp16 with fp32 PSUM accumulation.  x is
split hi/lo into two fp16 streams on the host (hi = fp16(x),
lo = (x-hi)*64) which gives ~fp29-equivalent precision where needed.
The router is computed in 3 passes (hi@Wr_hi + lo@Wr_hi/64 and a
separately-scaled hi@((Wr-Wr_hi)*256) correction) so that the worst-case
logit error (~4e-6) is far below the minimum top-2/top-3 gap (~3.9e-5)
=> bit-identical expert selection vs the fp32 reference.  The adapter
path (h and out GEMMs) runs 1-pass fp16: end-to-end rel err ~4e-4.

The kernel works on a host-transposed view of x (x^T, [D, tokens]) so
that the contraction dim D lands on SBUF partitions for every GEMM; the
output is produced in natural [tokens, D] layout.
"""

import numpy as np

# ---- problem constants (hardcoded per contract) ----
B, T, D, E, R = 2, 4096, 2048, 8, 16
BT = B * T                # 8192 tokens
NCORES = 8
TC = BT // NCORES         # 1024 tokens per core
MACRO = 512               # tokens per macro tile (one PSUM bank of fp32)
NMACRO = TC // MACRO      # 2
SUB = 128                 # tokens per sub tile (PE stationary width)
NSUB = MACRO // SUB       # 4
KC = D // 128             # 16 contraction chunks
ER = E * R                # 128 fused adapter width
LO_SCALE = 64.0           # x-lo stream stored as (x-hi)*64 (keeps fp16 normal)
WRL_SCALE = 256.0         # Wr-lo stored as (Wr-fp16(Wr))*256
NEG_BIG = -1.0e30

_CACHE = {}


def _split_multi_waits(nc):
    """This container's walrus rejects instructions carrying more than one
    sem-wait.  Hoist excess waits onto same-engine NOPs inserted just before
    the instruction (engine program order makes this equivalent)."""
    import concourse.mybir as mybir

    n_split = 0
    for f in nc.m.functions:
        for bb in f.blocks:
            insts = list(bb.instructions)
            out = []
            changed = False
            for ins in insts:
                si = ins.sync_info
                if si is not None and len(si.on_wait) > 1:
                    waits = list(si.on_wait)
                    for j, w in enumerate(waits[:-1]):
                        nop = mybir.InstNoOp(
                            name=f"{ins.name}-wsplit{j}", engine=ins.engine
                        )
                        nop.sync_info = mybir.SyncInfo(on_wait=[w], on_update=[])
                        out.append(nop)
                        n_split += 1
                    ins.sync_info = mybir.SyncInfo(
                        on_wait=[waits[-1]], on_update=list(si.on_update)
                    )
                    changed = True
                out.append(ins)
            if changed:
                bb.instructions = out
    return n_split


def _build_program(repeat=1):
    """Build the single-core SPMD Bass program (same NEFF on all 8 cores).

    repeat>1 builds a benchmarking variant that streams the same inputs
    through the whole pipeline `repeat` times (fresh DMAs each round) so the
    per-round steady-state time can be measured despite dispatch overhead.
    """
    import concourse.bass as bass
    import concourse.tile as tile
    import concourse.mybir as mybir

    dt = mybir.dt
    op = mybir.AluOpType
    AF = mybir.ActivationFunctionType

    nc = bass.Bass("TRN2", target_bir_lowering=False, debug=False, num_devices=1)

    # per-core DRAM tensors. x chunks pre-tiled on host: [p, k, t] with
    # element (d=128k+p, token t) so every DMA is contiguous per partition.
    xh_d = nc.dram_tensor("xh", [128, KC, TC], dt.float16, kind="ExternalInput").ap()
    xl_d = nc.dram_tensor("xl", [128, KC, TC], dt.float16, kind="ExternalInput").ap()
    wd_d = nc.dram_tensor("wd", [128, KC, ER], dt.float16, kind="ExternalInput").ap()
    wrh_d = nc.dram_tensor("wrh", [128, KC, E], dt.float16, kind="ExternalInput").ap()
    wrh64_d = nc.dram_tensor("wrh64", [128, KC, E], dt.float16, kind="ExternalInput").ap()
    wrl_d = nc.dram_tensor("wrl", [128, KC, E], dt.float16, kind="ExternalInput").ap()
    wu_d = nc.dram_tensor("wu", [ER, D], dt.float16, kind="ExternalInput").ap()
    brb_d = nc.dram_tensor("brb", [128, E], dt.float32, kind="ExternalInput").ap()
    ident_d = nc.dram_tensor("ident", [128, 128], dt.float32, kind="ExternalInput").ap()
    out_d = nc.dram_tensor("out", [TC, D], dt.float32, kind="ExternalOutput").ap()

    with tile.TileContext(nc) as tc:
        with (
            tc.tile_pool(name="consts", bufs=1) as cpool,
            tc.tile_pool(name="xdata", bufs=(1 if repeat == 1 else 2)) as xpool,
            tc.tile_pool(name="work", bufs=2) as wk,
            tc.tile_pool(name="outsb", bufs=4) as osb,
            tc.tile_pool(name="ps_l", bufs=1, space="PSUM") as ps_l,
            tc.tile_pool(name="ps_c", bufs=1, space="PSUM") as ps_c,
            tc.tile_pool(name="ps_h", bufs=2, space="PSUM") as ps_h,
            tc.tile_pool(name="ps_t", bufs=1, space="PSUM") as ps_t,
            tc.tile_pool(name="ps_w", bufs=1, space="PSUM") as ps_w,
            tc.tile_pool(name="ps_o", bufs=2, space="PSUM") as ps_o,
        ):
            # ---- constants / weights ----
            wd_sb = cpool.tile([128, KC, ER], dt.float16)
            nc.sync.dma_start(wd_sb[:], wd_d[:])
            wrh_sb = cpool.tile([128, KC, E], dt.float16)
            nc.sync.dma_start(wrh_sb[:], wrh_d[:])
            wrh64_sb = cpool.tile([128, KC, E], dt.float16)
            nc.sync.dma_start(wrh64_sb[:], wrh64_d[:])
            wrl_sb = cpool.tile([128, KC, E], dt.float16)
            nc.sync.dma_start(wrl_sb[:], wrl_d[:])
            wu_sb = cpool.tile([ER, D], dt.float16)
            nc.sync.dma_start(wu_sb[:], wu_d[:])
            brb_sb = cpool.tile([128, E], dt.float32)
            nc.sync.dma_start(brb_sb[:], brb_d[:])
            ident_sb = cpool.tile([128, 128], dt.float32)
            nc.sync.dma_start(ident_sb[:], ident_d[:])

            for rep in range(repeat):
              # ---- x streams (whole core slice resident; chunked DMAs) ----
              xh_sb = xpool.tile([128, KC, TC], dt.float16)
              xl_sb = xpool.tile([128, KC, TC], dt.float16)
              for k in range(KC):
                nc.sync.dma_start(xh_sb[:, k, :], xh_d[:, k, :])
              for k in range(KC):
                nc.sync.dma_start(xl_sb[:, k, :], xl_d[:, k, :])

              for m in range(NMACRO):
                ts = slice(m * MACRO, (m + 1) * MACRO)

                with nc.named_scope(f"router_mm_{m}"):
                    # main: hi @ Wr_hi + (lo*64) @ (Wr_hi/64)
                    psum_l = ps_l.tile([E, MACRO], dt.float32)
                    for k in range(KC):
                        nc.tensor.matmul(
                            psum_l[:], wrh_sb[:, k, :], xh_sb[:, k, ts],
                            start=(k == 0), stop=False,
                        )
                    for k in range(KC):
                        nc.tensor.matmul(
                            psum_l[:], wrh64_sb[:, k, :], xl_sb[:, k, ts],
                            start=False, stop=(k == KC - 1),
                        )
                    # correction: hi @ (Wr_lo*256), rescaled on evac
                    psum_c = ps_c.tile([E, MACRO], dt.float32)
                    for k in range(KC):
                        nc.tensor.matmul(
                            psum_c[:], wrl_sb[:, k, :], xh_sb[:, k, ts],
                            start=(k == 0), stop=(k == KC - 1),
                        )
                    corr_sb = wk.tile([E, MACRO], dt.float32)
                    nc.scalar.mul(corr_sb[:], psum_c[:], 1.0 / WRL_SCALE)
                    lT = wk.tile([E, MACRO], dt.float32)
                    nc.vector.tensor_add(lT[:], psum_l[:], corr_sb[:])

                with nc.named_scope(f"down_mm_{m}"):
                    psum_h = ps_h.tile([ER, MACRO], dt.float32)
                    for k in range(KC):
                        nc.tensor.matmul(
                            psum_h[:], wd_sb[:, k, :], xh_sb[:, k, ts],
                            start=(k == 0), stop=(k == KC - 1),
                        )

                with nc.named_scope(f"routing_{m}"):
                    # stack [E, MACRO] -> [E*NSUB, SUB] then one PE transpose
                    lT_st = wk.tile([E * NSUB, SUB], dt.float32)
                    for s in range(NSUB):
                        nc.sync.dma_start(
                            lT_st[s * E:(s + 1) * E, :], lT[:, s * SUB:(s + 1) * SUB]
                        )
                    psum_lt = ps_t.tile([128, NSUB * E], dt.float32)
                    nc.tensor.transpose(
                        psum_lt[:], lT_st[:], ident_sb[: E * NSUB, : E * NSUB]
                    )
                    # logits [tok=128, s, e] with router bias
                    l_all = wk.tile([128, NSUB, E], dt.float32)
                    brb_b = brb_sb[:].unsqueeze(1).broadcast_to([128, NSUB, E])
                    nc.vector.tensor_add(
                        l_all[:], psum_lt[:].rearrange("p (s e) -> p s e", e=E), brb_b
                    )
                    v1 = wk.tile([128, NSUB], dt.float32)
                    nc.vector.reduce_max(v1[:], l_all[:], axis=mybir.AxisListType.X)
                    v1b = v1[:].unsqueeze(-1).broadcast_to([128, NSUB, E])
                    eq = wk.tile([128, NSUB, E], dt.float32)
                    nc.vector.tensor_tensor(eq[:], l_all[:], v1b, op.is_equal)
                    lm = wk.tile([128, NSUB, E], dt.float32)
                    nc.vector.scalar_tensor_tensor(
                        lm[:], eq[:], NEG_BIG, l_all[:], op0=op.mult, op1=op.add
                    )
                    v2 = wk.tile([128, NSUB], dt.float32)
                    nc.vector.reduce_max(v2[:], lm[:], axis=mybir.AxisListType.X)
                    t1 = wk.tile([128, NSUB, E], dt.float32)
                    nc.vector.tensor_sub(t1[:], l_all[:], v1b)
                    e1 = wk.tile([128, NSUB, E], dt.float32)
                    nc.scalar.activation(e1[:], t1[:], AF.Exp)
                    v2b = v2[:].unsqueeze(-1).broadcast_to([128, NSUB, E])
                    m2 = wk.tile([128, NSUB, E], dt.float32)
                    nc.vector.tensor_tensor(m2[:], l_all[:], v2b, op.is_ge)
                    num = wk.tile([128, NSUB, E], dt.float32)
                    nc.vector.tensor_mul(num[:], e1[:], m2[:])
                    den = wk.tile([128, NSUB], dt.float32)
                    nc.vector.reduce_sum(den[:], num[:], axis=mybir.AxisListType.X)
                    rec = wk.tile([128, NSUB], dt.float32)
                    nc.vector.reciprocal(rec[:], den[:])
                    recb = rec[:].unsqueeze(-1).broadcast_to([128, NSUB, E])
                    w_all = wk.tile([128, NSUB, E], dt.float32)
                    nc.vector.tensor_mul(w_all[:], num[:], recb)

                with nc.named_scope(f"scale_up_{m}"):
                    g = wk.tile([ER, MACRO], dt.float16)
                    for s in range(NSUB):
                        # expand w over rank (free bcast), transpose to [j, t]
                        wF = wk.tile([128, E, R], dt.float32)
                        nc.vector.tensor_copy(
                            wF[:], w_all[:, s, :].unsqueeze(-1).broadcast_to([128, E, R])
                        )
                        psum_w = ps_w.tile([128, 128], dt.float32)
                        nc.tensor.transpose(
                            psum_w[:],
                            wF[:].rearrange("p e r -> p (e r)"),
                            ident_sb[:],
                        )
                        wexp = wk.tile([128, SUB], dt.float32)
                        nc.scalar.copy(wexp[:], psum_w[:])
                        # g = relu(h) * w   (w >= 0 so relu(h*w) == relu(h)*w)
                        nc.vector.scalar_tensor_tensor(
                            g[:, s * SUB:(s + 1) * SUB],
                            psum_h[:, s * SUB:(s + 1) * SUB],
                            0.0,
                            wexp[:],
                            op0=op.max,
                            op1=op.mult,
                        )
                    for s in range(NSUB):
                        row0 = m * MACRO + s * SUB
                        for dc in range(4):
                            psum_o = ps_o.tile([SUB, 512], dt.float32)
                            nc.tensor.matmul(
                                psum_o[:],
                                g[:, s * SUB:(s + 1) * SUB],
                                wu_sb[:, dc * 512:(dc + 1) * 512],
                                start=True, stop=True,
                            )
                            ob = osb.tile([SUB, 512], dt.float32)
                            if dc % 2 == 0:
                                nc.vector.tensor_copy(ob[:], psum_o[:])
                            else:
                                nc.scalar.copy(ob[:], psum_o[:])
                            nc.sync.dma_start(
                                out_d[row0:row0 + SUB, dc * 512:(dc + 1) * 512], ob[:]
                            )
    _split_multi_waits(nc)
    return nc


def _prep_inputs(x, Wr, br, Wd, Wu):
    """Host-side layout prep + sharding. Returns list of per-core in_maps."""
    f16, f32 = np.float16, np.float32
    xf = np.ascontiguousarray(x.reshape(BT, D).T)          # [D, BT] f32
    xh = xf.astype(f16)
    xl = ((xf - xh.astype(f32)) * LO_SCALE).astype(f16)

    W1 = np.ascontiguousarray(Wd.transpose(1, 0, 2).reshape(D, ER))  # [D, 128]
    wrh = Wr.astype(f16)
    wrh64 = (wrh.astype(f32) / LO_SCALE).astype(f16)
    wrl = ((Wr - wrh.astype(f32)) * WRL_SCALE).astype(f16)

    def chunkify(a, width):  # [D, width] -> [128, KC, width]
        return np.ascontiguousarray(
            a.reshape(KC, 128, width).transpose(1, 0, 2)
        )

    wd_t = chunkify(W1.astype(f16), ER)
    wrh_t = chunkify(wrh, E)
    wrh64_t = chunkify(wrh64, E)
    wrl_t = chunkify(wrl, E)
    wu_t = np.ascontiguousarray(Wu.reshape(ER, D).astype(f16))
    brb = np.ascontiguousarray(np.tile(br.astype(f32), (128, 1)))
    ident = np.eye(128, dtype=f32)

    in_maps = []
    for c in range(NCORES):
        sl = slice(c * TC, (c + 1) * TC)
        in_maps.append({
            "xh": chunkify(xh[:, sl], TC),
            "xl": chunkify(xl[:, sl], TC),
            "wd": wd_t,
            "wrh": wrh_t,
            "wrh64": wrh64_t,
            "wrl": wrl_t,
            "wu": wu_t,
            "brb": brb,
            "ident": ident,
        })
    return in_maps


def _get_program(repeat=1):
    key = ("nc", repeat)
    if key not in _CACHE:
        _CACHE[key] = _build_program(repeat)
    return _CACHE[key]


def run_on_device(in_maps, repeat=1, **kwargs):
    from concourse import bass_utils
    nc = _get_program(repeat)
    return bass_utils.run_bass_kernel_spmd(
        nc, in_maps, core_ids=list(range(NCORES)), **kwargs
    )


def kernel(x, Wr, br, Wd, bd, Wu, bu, **_ignored):
    x = np.asarray(x, dtype=np.float32)
    in_maps = _prep_inputs(
        x,
        np.asarray(Wr, dtype=np.float32),
        np.asarray(br, dtype=np.float32),
        np.asarray(Wd, dtype=np.float32),
        np.asarray(Wu, dtype=np.float32),
    )
    res = run_on_device(in_maps)
    out = np.concatenate([r["out"] for r in res.results], axis=0)
    return out.reshape(B, T, D)
